# revision 1
# baseline (speedup 1.0000x reference)
"""Trainium2 Bass kernel for BotanHadamardTransform: y = x @ H, with
x [4, 4096, 4096] f32 and H [4096, 4096] f32 the normalized Sylvester
Hadamard matrix H_4096 / 64.

Algorithm: Sylvester Hadamard matrices factor as Kronecker products,
H_4096 = H_A (x) H_B with A*B = 4096. For a row vector v (len 4096),
v @ H_4096 = FWHT_A applied across the A axis of (v.reshape(A, B) @ H_B).
This reduces per-row work from O(n^2) to O(n*(B + log2 A)).

Mapping to hardware (per core, 1/8 of the 16384 rows = 2048 rows):
  - host pre-transposes x so the device sees xT [4096, 2048] with the
    contraction dim on partitions (natural matmul layout, no on-device
    transposes)
  - PE contracts the low B=512 of each k-index against Hf = H[0:512,0:512]
    (which equals H_512/64 exactly) as fp32r matmuls, N=512 moving columns
  - the high A=8 factor is a 3-stage FWHT butterfly across 128-partition
    chunks; stage 1 runs fused with PSUM eviction (ScalarE evicts one
    accumulator, VectorE adds/subs against the other still in PSUM);
    stages 2-3 are whole-block VectorE ops with fully contiguous access
    patterns, with an optional thin GpSimd chunk slice
  - output is written transposed (yT [4096, 2048]); host transposes back

Buffer scheme per r-tile (R=512 moving columns, 4 r-tiles per core):
  G1 blocks (xinb, f32 [128,8,512]): DMA-in dest; dead after rounding;
     reused as stage-1 output (the butterfly ping); s2 reads them.
  xr blocks (f32r): rounded matmul input; dead after matmuls; slots
     reused for stage-2 outputs (f32 bitcast view); s3 reads those.
  s3 writes fresh G1-pool blocks; DMA-out drains them.
"""
import os
import sys

sys.path.insert(0, "/opt/trn_rl_repo")

import numpy as np

import concourse.bass as bass  # noqa: F401
import concourse.tile as tile
from concourse import bacc, mybir
from concourse.bass_utils import run_bass_kernel_spmd

N_CORES = 8
N = 4096            # hidden dim
ROWS = 4 * 4096     # total rows
RC = ROWS // N_CORES  # rows (columns of xT) per core = 2048

B = 512             # PE-contracted Kronecker factor (Hf = H_512/64)
R = 512             # moving columns per r-tile

A = N // B               # butterfly factor (8)
SUB = B // 128           # accumulating matmuls per output chunk (4)
NCH = N // 128           # 32 chunks of 128 partitions
BCH = 2 * SUB            # chunks per pair-block (8)
NPAIR = A // 2           # pair blocks (4)
QH = 2                   # q-values per PSUM half-block


def _build():
    nc = bacc.Bacc("TRN2", target_bir_lowering=False, debug=False,
                   num_devices=N_CORES)
    xT_ap = nc.dram_tensor("xT", [N, RC], mybir.dt.float32,
                           kind="ExternalInput").ap()
    hf_ap = nc.dram_tensor("Hf", [B, B], mybir.dt.float32,
                           kind="ExternalInput").ap()
    yT_ap = nc.dram_tensor("yT", [N, RC], mybir.dt.float32,
                           kind="ExternalOutput").ap()

    f32 = mybir.dt.float32
    f32r = mybir.dt.float32r

    xT_v = xT_ap.rearrange("(c p) r -> p c r", p=128)   # [128, NCH, RC]
    yT_v = yT_ap.rearrange("(c p) r -> p c r", p=128)

    n_rt = RC // R

    with tile.TileContext(nc) as tc:
        with (
            tc.tile_pool(name="hfp", bufs=1) as hfp,
            tc.tile_pool(name="xbin", bufs=2) as xbinp,
            tc.tile_pool(name="xr", bufs=2) as xrp,
            tc.tile_pool(name="g13", bufs=5) as g13p,
            tc.tile_pool(name="g2", bufs=3) as g2p,
            tc.tile_pool(name="ev", bufs=1) as evp,
            tc.tile_pool(name="ps", bufs=2, space="PSUM") as psp,
        ):
            # stationary Hf: stage f32 via an xr-pool slot, round to f32r.
            # layout: hf[p, s*B + col] = Hf[s*128 + p, col]
            hf_st = xrp.tile([128, SUB * B], f32, tag="xr", name="hf_stage")
            for s in range(SUB):
                nc.sync.dma_start(hf_st[:, s * B:(s + 1) * B],
                                  hf_ap[s * 128:(s + 1) * 128, :])
            hf_mm = hfp.tile([128, SUB * B], f32r, tag="hfr")
            nc.scalar.copy(hf_mm[:], hf_st[:])

            def hf_block(s, q):
                # lhsT block [k=128 (i2 sub s), m=128 (j2 sub q)]
                return hf_mm[:, s * B + q * 128: s * B + q * 128 + 128]

            def bf_pair(dst_add, dst_sub, src0, src1, gp_ch=2):
                """dst_add = src0+src1, dst_sub = src0-src1 on [128,BCH,R]
                tiles. GpSimd takes the trailing gp_ch chunks of each op
                (measured costs: DVE ~0.8us + 0.7us/chunk per op, GpSimd
                ~3.5us + 1.0us/chunk -> 2 chunks balances the two engines
                at ~10us per pair), VectorE the rest; both run in parallel
                with fully contiguous access patterns."""
                c_gp = BCH - gp_ch
                for (eng, c0, c1) in (("v", 0, c_gp), ("g", c_gp, BCH)):
                    if c0 >= c1:
                        continue
                    sl = lambda t: t[:, c0:c1, :].rearrange("p c r -> p (c r)")
                    if eng == "v":
                        nc.vector.tensor_add(sl(dst_add), sl(src0), sl(src1))
                        nc.vector.tensor_sub(sl(dst_sub), sl(src0), sl(src1))
                    else:
                        nc.gpsimd.tensor_add(sl(dst_add), sl(src0), sl(src1))
                        nc.gpsimd.tensor_sub(sl(dst_sub), sl(src0), sl(src1))

            for it in range(n_rt):
                r0 = it * R
                g1 = []   # stage-1 output tiles
                for m in range(NPAIR):
                    ch0 = m * BCH
                    xb = xbinp.tile([128, BCH, R], f32, tag="xbin",
                                    name=f"xb_{it}_{m}")
                    g1m = g13p.tile([128, BCH, R], f32, tag="g13",
                                    name=f"g1_{it}_{m}")
                    g1.append(g1m)
                    nc.sync.dma_start(xb[:],
                                      xT_v[:, ch0:ch0 + BCH, r0:r0 + R])
                    # rounding pass f32 -> f32r (ScalarE); xb is dead after
                    # this and becomes the stage-1 destination
                    xg = xrp.tile([128, BCH, R], f32r, tag="xr",
                                  name=f"xg_{it}_{m}")
                    nc.scalar.copy(xg[:], xb[:])

                    for qh in range(SUB // QH):
                        pg = [psp.tile([128, QH * R], f32, tag=f"pg{j}",
                                       name=f"pg{j}_{it}_{m}_{qh}")
                              for j in range(2)]
                        for qq in range(QH):
                            q = qh * QH + qq
                            for s in range(SUB):
                                for j in range(2):
                                    nc.tensor.matmul(
                                        pg[j][:, qq * R:(qq + 1) * R],
                                        hf_block(s, q),
                                        xg[:, j * SUB + s, :],
                                        start=(s == 0),
                                        stop=(s == SUB - 1),
                                    )
                        # stage-1 butterfly fused with eviction: ScalarE
                        # evicts pg0 to a scratch tile, VectorE adds/subs
                        # against pg1 still in PSUM (DVE has one PSUM port)
                        ev = evp.tile([128, QH * R], f32, tag="ev",
                                      name=f"ev_{it}_{m}_{qh}")
                        nc.scalar.copy(ev[:], pg[0][:])
                        ca = qh * QH
                        cb = SUB + qh * QH
                        oa = g1m[:, ca:ca + QH, :].rearrange(
                            "p c r -> p (c r)")
                        ob = g1m[:, cb:cb + QH, :].rearrange(
                            "p c r -> p (c r)")
                        nc.vector.tensor_add(oa, ev[:], pg[1][:])
                        nc.vector.tensor_sub(ob, ev[:], pg[1][:])

                # remaining stages: block-pair adds; xr slots freed by the
                # matmuls become the f32 destinations via fresh pool tiles
                if A == 4:
                    g2 = [g2p.tile([128, BCH, R], f32, tag="g2",
                                   name=f"g2_{it}_{i}") for i in range(2)]
                    bf_pair(g2[0], g2[1], g1[0], g1[1], 2)
                    for i in range(2):
                        nc.scalar.dma_start(
                            yT_v[:, i * BCH:(i + 1) * BCH, r0:r0 + R],
                            g2[i][:])
                else:  # A == 8
                    g2 = [g2p.tile([128, BCH, R], f32, tag="g2",
                                   name=f"g2_{it}_{i}") for i in range(4)]
                    bf_pair(g2[0], g2[1], g1[0], g1[1], 2)
                    bf_pair(g2[2], g2[3], g1[2], g1[3], 2)

                    # stage 3: outputs in final chunk order
                    g3 = [g13p.tile([128, BCH, R], f32, tag="g13",
                                   name=f"g3_{it}_{i}") for i in range(4)]
                    bf_pair(g3[0], g3[2], g2[0], g2[2], 2)
                    bf_pair(g3[1], g3[3], g2[1], g2[3], 2)

                    for i in range(4):
                        nc.scalar.dma_start(
                            yT_v[:, i * BCH:(i + 1) * BCH, r0:r0 + R],
                            g3[i][:])

    nc.compile()
    return nc


_prog = None


def _get_prog():
    global _prog
    if _prog is None:
        _prog = _build()
    return _prog


def _run(xT, Hf, trace=False):
    nc = _get_prog()
    in_maps = [
        {"xT": np.ascontiguousarray(xT[:, c * RC:(c + 1) * RC]), "Hf": Hf}
        for c in range(N_CORES)
    ]
    res = run_bass_kernel_spmd(nc, in_maps, core_ids=list(range(N_CORES)),
                               trace=trace)
    return res


def kernel(x, H):
    x = np.asarray(x)
    H = np.asarray(H)
    xT = np.ascontiguousarray(x.reshape(ROWS, N).T)          # [N, ROWS]
    Hf = np.ascontiguousarray(H[:B, :B])                      # = H_B / 64
    res = _run(xT, Hf)
    y = np.empty((ROWS, N), dtype=np.float32)
    for c in range(N_CORES):
        y[c * RC:(c + 1) * RC, :] = res.results[c]["yT"].T
    return y.reshape(4, 4096, N)



# revision 4
# speedup vs baseline: 2.2235x; 2.2235x over previous
"""Trainium2 Bass kernel for BotanHadamardTransform: y = x @ H, with
x [4, 4096, 4096] f32 and H [4096, 4096] f32 the normalized Sylvester
Hadamard matrix H_4096 / 64.

Algorithm (bf16 end-to-end, rel err ~4e-3 vs the 2e-2 gate):
Sylvester Hadamard matrices factor as Kronecker products,
H_4096 = H_8 (x) H_512.  For a row v (len 4096) viewed as [a=8, b=512]:
  1. FWHT over the a axis (3 butterfly stages of add/sub) -- done input-
     side (decimation-in-time) on bf16 SBUF tiles at DVE 2x mode,
  2. per-a matmul with Hf = H[:512,:512] (= H_512/64, exact in bf16)
     on the PE at bf16 rate, f32 PSUM accumulation,
  3. ScalarE evicts PSUM f32 -> bf16 SBUF, DMA out.

Data-parallel over 8 cores: core c owns 2048 rows.  The host packs per-
core inputs as pre-tiled bf16 blocks so every DMA is a single fully
contiguous 2 MiB transfer:
  xt [NT, 128, 32, R]: xt[t, p, c, i] = x_rows.T[c*128 + p, t*R + i]
  yt [NT, 128, 2, 4, 4, R]: y[row = t*R + i,
     col = 2048*g + 512*jj + 128*q + p] = yt[t, p, g, q, jj, i]
Per r-tile: butterflies chunk-pair 16/8/4-chunk blocks (a lives in the
k-chunk index: k = a*512 + s*128 + p, chunk = a*4 + s); matmuls contract
k in 4 accumulating steps per output chunk, grouped 4 a'-slices per
2-bank PSUM tile so ScalarE evictions are FD-1024 single instructions.
"""
import sys

sys.path.insert(0, "/opt/trn_rl_repo")

import numpy as np
import ml_dtypes

import concourse.bass as bass  # noqa: F401
import concourse.tile as tile
from concourse import bacc, mybir
from concourse.bass_utils import run_bass_kernel_spmd

BF16 = ml_dtypes.bfloat16

N_CORES = 8
N = 4096             # hidden dim
ROWS = 4 * 4096      # total rows
RC = ROWS // N_CORES  # rows per core = 2048

B = 512              # PE-contracted Kronecker factor (Hf = H_512/64)
A = N // B           # butterfly factor (8)
NCH = N // 128       # 32 k-chunks
SUB = B // 128       # accumulating matmuls per output chunk (4)
R = 256              # moving columns per r-tile
NT = RC // R         # r-tiles per core


def _build():
    nc = bacc.Bacc("TRN2", target_bir_lowering=False, debug=False,
                   num_devices=N_CORES)
    bf = mybir.dt.bfloat16
    f32 = mybir.dt.float32

    x_ap = nc.dram_tensor("xt", [NT, 128, NCH, R], bf,
                          kind="ExternalInput").ap()
    hf_ap = nc.dram_tensor("hf", [128, SUB * B], bf,
                           kind="ExternalInput").ap()
    y_ap = nc.dram_tensor("yt", [NT, 128, 2, SUB, SUB, R], bf,
                          kind="ExternalOutput").ap()

    with tile.TileContext(nc) as tc:
        with (
            tc.tile_pool(name="hfp", bufs=1) as hfp,
            tc.tile_pool(name="xb", bufs=3) as xbp,
            tc.tile_pool(name="g1", bufs=2) as g1p,
            tc.tile_pool(name="g2", bufs=2) as g2p,
            tc.tile_pool(name="g3", bufs=2) as g3p,
            tc.tile_pool(name="yb", bufs=2) as ybp,
            tc.tile_pool(name="ps", bufs=4, space="PSUM") as psp,
        ):
            hf = hfp.tile([128, SUB * B], bf, tag="hf")
            nc.sync.dma_start(hf[:], hf_ap)

            def hfblk(s, q):
                o = s * B + q * 128
                return hf[:, o:o + 128]

            for t in range(NT):
                xb = xbp.tile([128, NCH, R], bf, tag="xb", name=f"xb{t}")
                nc.sync.dma_start(xb[:], x_ap[t])
                g1 = g1p.tile([128, NCH, R], bf, tag="g1", name=f"g1{t}")
                g2 = g2p.tile([128, NCH, R], bf, tag="g2", name=f"g2{t}")
                g3 = g3p.tile([128, NCH, R], bf, tag="g3", name=f"g3{t}")

                def fl(tt, c0, c1):
                    return tt[:, c0:c1, :].rearrange("p c r -> p (c r)")

                # stage 1: a-stride 4 (16-chunk blocks)
                nc.vector.tensor_add(fl(g1, 0, 16), fl(xb, 0, 16),
                                     fl(xb, 16, 32))
                nc.vector.tensor_sub(fl(g1, 16, 32), fl(xb, 0, 16),
                                     fl(xb, 16, 32))
                # stage 2: a-stride 2 (8-chunk blocks)
                for h in (0, 16):
                    nc.vector.tensor_add(fl(g2, h, h + 8), fl(g1, h, h + 8),
                                         fl(g1, h + 8, h + 16))
                    nc.vector.tensor_sub(fl(g2, h + 8, h + 16),
                                         fl(g1, h, h + 8),
                                         fl(g1, h + 8, h + 16))
                # stage 3: a-stride 1 (4-chunk blocks)
                for m in range(4):
                    h = 8 * m
                    nc.vector.tensor_add(fl(g3, h, h + 4), fl(g2, h, h + 4),
                                         fl(g2, h + 4, h + 8))
                    nc.vector.tensor_sub(fl(g3, h + 4, h + 8),
                                         fl(g2, h, h + 4),
                                         fl(g2, h + 4, h + 8))

                yb = ybp.tile([128, 2, SUB, SUB, R], bf, tag="yb",
                              name=f"yb{t}")
                for g in range(2):
                    for q in range(SUB):
                        ps = psp.tile([128, SUB, R], f32, tag="ps",
                                      name=f"ps{t}_{g}_{q}")
                        for s in range(SUB):
                            # each matmul writes one FULL psum bank (512
                            # f32): a start=True clears has_written for the
                            # whole bank, so accumulation regions must be
                            # bank-sized.  Bank u holds the a'-pair
                            # (4g+2u, 4g+2u+1); their g3 chunks sit SUB
                            # apart, taken via a stride-SUB slice.
                            for u in range(2):
                                c0 = (SUB * g + 2 * u) * SUB + s
                                nc.tensor.matmul(
                                    ps[:, 2 * u:2 * u + 2, :], hfblk(s, q),
                                    g3[:, c0:c0 + SUB + 1:SUB, :],
                                    start=(s == 0), stop=(s == SUB - 1))
                        nc.scalar.copy(
                            yb[:, g, q, :, :].rearrange("p a r -> p (a r)"),
                            ps[:].rearrange("p a r -> p (a r)"))
                nc.scalar.dma_start(y_ap[t], yb[:])

    nc.compile()
    return nc


_prog = None


def _get_prog():
    global _prog
    if _prog is None:
        _prog = _build()
    return _prog


def prepare_in_maps(x, H):
    """Host-side pack: per-core pre-tiled bf16 blocks."""
    x = np.asarray(x, dtype=np.float32).reshape(ROWS, N)
    Hf = np.asarray(H, dtype=np.float32)[:B, :B].astype(BF16)
    hf_dev = np.ascontiguousarray(
        Hf.reshape(SUB, 128, B).transpose(1, 0, 2).reshape(128, SUB * B))
    in_maps = []
    for c in range(N_CORES):
        xc = x[c * RC:(c + 1) * RC].astype(BF16)        # [RC, N]
        xt = xc.T.reshape(NCH, 128, NT, R).transpose(2, 1, 0, 3)
        in_maps.append({"xt": np.ascontiguousarray(xt), "hf": hf_dev})
    return in_maps


def _run(in_maps, trace=False):
    nc = _get_prog()
    return run_bass_kernel_spmd(nc, in_maps, core_ids=list(range(N_CORES)),
                                trace=trace)


def kernel(x, H):
    res = _run(prepare_in_maps(x, H))
    y = np.empty((ROWS, N), dtype=np.float32)
    for c in range(N_CORES):
        yt = res.results[c]["yt"]                        # [NT,128,2,4,4,R]
        yc = yt.transpose(0, 5, 2, 4, 3, 1).reshape(RC, N)
        y[c * RC:(c + 1) * RC] = yc.astype(np.float32)
    return y.reshape(4, 4096, N)
